# revision 1
# baseline (speedup 1.0000x reference)
"""Trainium2 Bass kernel for Box2FeatureGeneratorV2 — sparse span edition.

Key ideas vs the dense baseline:
  1. feat_sum is EXACTLY zero outside the rasterized boxes, so away from
     boxes every layer's activation equals a per-channel constant
     ("background"), computable on the host from the weights alone.
     Each conv layer therefore only computes spans covering
     (box support dilated by k+1) plus the H/W border frames; everything
     else is filled with the background constant (exact, not approximate).
  2. The span schedule is data-dependent, so kernel() compiles one
     specialized program PER CORE per box layout (cached), and dispatches
     the 8 single-core programs concurrently on the 8 NeuronCores.
  3. Raster edge tests use exact 3-way bf16 splits of the edge
     coefficients against integer-centered cell coords (exact in bf16):
     K=10 bf16 matmuls run at 1 cycle/row instead of fp32's 4.
  4. Slab widths are load-balanced per call from the span plan.
"""

import sys
import numpy as np

sys.path.insert(0, "/opt/trn_rl_repo")

H, W, C, NBOX = 200, 704, 256, 128
NCORES = 8
HALO = 6
HL = H + 2                  # 202 buffer rows (1 zero row each side)
DOFF = 4
RT_N = 505                  # raster tile free size
XMIN, YMIN, DX, DY = -140.8, -40.0, 0.4, 0.4
BN_EPS = 1e-5
MAXW = 96                   # max slab width (SBUF limit)
MINW = 64
GAP = 4                     # merge spans with gaps <= GAP
SPAN_MAX = 102              # max span width (PSUM bank: 5*102=510 fp32)


# ---------------------------------------------------------------------------
# host-side planning
# ---------------------------------------------------------------------------

def _inside_mask(pred_box):
    """[H, W] bool: cells inside any box (mirrors reference fp32 math)."""
    f32 = np.float32
    gx = ((pred_box[:, :4, 0] - XMIN) / DX).astype(f32)
    gy = ((pred_box[:, :4, 1] - YMIN) / DY).astype(f32)
    cxs = (np.arange(W, dtype=f32) + 0.5)
    cys = (np.arange(H, dtype=f32) + 0.5)
    inside = np.ones((NBOX, H, W), bool)
    for e in range(4):
        ax, ay = gx[:, e], gy[:, e]
        bx, by = gx[:, (e + 1) % 4], gy[:, (e + 1) % 4]
        vx, vy = bx - ax, by - ay
        c = (vx[:, None, None] * (cys[None, :, None] - ay[:, None, None])
             - vy[:, None, None] * (cxs[None, None, :] - ax[:, None, None]))
        inside &= (c >= 0)
    return inside.any(0)


def _dilate(m):
    out = m.copy()
    out[:-1] |= m[1:]
    out[1:] |= m[:-1]
    out2 = out.copy()
    out2[:, :-1] |= out[:, 1:]
    out2[:, 1:] |= out[:, :-1]
    return out2


def _runs(colact, gap, max_w):
    runs = []
    in_run = False
    start = 0
    for c in range(len(colact)):
        if colact[c] and not in_run:
            start = c
            in_run = True
        elif not colact[c] and in_run:
            runs.append([start, c])
            in_run = False
    if in_run:
        runs.append([start, len(colact)])
    merged = []
    for s, e in runs:
        if merged and s - merged[-1][1] <= gap:
            merged[-1][1] = e
        else:
            merged.append([s, e])
    out = []
    for s, e in merged:
        w = e - s
        nsub = (w + max_w - 1) // max_w
        for j in range(nsub):
            s0 = s + j * max_w
            out.append((s0, min(max_w, e - s0)))
    return out


def _make_plans(pred_box):
    support = _inside_mask(pred_box)
    occ = _dilate(support)      # +1 safety margin for the raster spans
    acts = []
    a = support
    for k in range(6):
        a = _dilate(a)
        ab = a.copy()
        d = k + 1
        ab[:d] = True
        ab[-d:] = True
        ab[:, :d] = True
        ab[:, -d:] = True
        acts.append(ab)

    bounds = _balance_bounds(acts)

    plans = []
    for core in range(NCORES):
        w0, w1 = bounds[core], bounds[core + 1]
        Wc = w1 - w0
        WL = Wc + 2 * HALO
        spans = []           # [k][t] -> list[(c0,w)]
        fills = []           # [k][t] -> list[(c0,w)] complement (odd k) or full
        for k in range(6):
            ab = acts[k]
            ksp, kfl = [], []
            lo_map = w0 - HALO          # map col of buffer col 0
            vlo = k + 1
            vhi = WL - (k + 1)
            ilo = max(vlo, -lo_map)              # in-map & valid window
            ihi = min(vhi, W - lo_map)
            for t in range(40):
                sl = ab[5 * t:5 * t + 5, max(0, lo_map):min(W, lo_map + WL)]
                colact = np.zeros(WL, bool)
                colact[max(0, -lo_map):max(0, -lo_map) + sl.shape[1]] = sl.any(0)
                colact[:ilo] = False
                colact[ihi:] = False
                rs = _runs(colact, GAP, SPAN_MAX)
                ksp.append(rs)
                # complement runs within [ilo, ihi)
                cf = []
                pos = ilo
                for s, w in rs:
                    if s > pos:
                        cf.append((pos, s - pos))
                    pos = s + w
                if ihi > pos:
                    cf.append((pos, ihi - pos))
                kfl.append(cf)
            spans.append(ksp)
            fills.append(kfl)
        rspans = []
        lo_map = w0 - HALO
        ilo = max(0, -lo_map)
        ihi = min(WL, W - lo_map)
        for t in range(40):
            sl = occ[5 * t:5 * t + 5, max(0, lo_map):min(W, lo_map + WL)]
            colact = np.zeros(WL, bool)
            colact[max(0, -lo_map):max(0, -lo_map) + sl.shape[1]] = sl.any(0)
            colact[:ilo] = False
            colact[ihi:] = False
            rspans.append(_runs(colact, 6, SPAN_MAX))
        plans.append(dict(w0=w0, W=Wc, WL=WL, spans=spans, fills=fills,
                          rspans=rspans))
    return plans


def _slab_cost(acts, w0, w1):
    """Analytic per-core cost (ns-ish) for slab [w0, w1)."""
    Wc = w1 - w0
    WL = Wc + 2 * HALO
    lo_map = w0 - HALO
    cost = 0.0
    for k in range(6):
        ab = acts[k]
        vlo, vhi = k + 1, WL - (k + 1)
        ilo = max(vlo, -lo_map)
        ihi = min(vhi, W - lo_map)
        for t in range(40):
            sl = ab[5 * t:5 * t + 5, max(0, lo_map):min(W, lo_map + WL)]
            colact = np.zeros(WL, bool)
            colact[max(0, -lo_map):max(0, -lo_map) + sl.shape[1]] = sl.any(0)
            colact[:ilo] = False
            colact[ihi:] = False
            for s, w in _runs(colact, GAP, SPAN_MAX):
                cost += 2 * 18 * (5 * w * 0.4167 + 24)
    occ0 = acts[0]
    ilo = max(0, -lo_map)
    ihi = min(WL, W - lo_map)
    for t in range(40):
        sl = occ0[5 * t:5 * t + 5, max(0, lo_map):min(W, lo_map + WL)]
        colact = np.zeros(WL, bool)
        colact[max(0, -lo_map):max(0, -lo_map) + sl.shape[1]] = sl.any(0)
        colact[:ilo] = False
        colact[ihi:] = False
        for s, w in _runs(colact, 6, SPAN_MAX):
            cost += 7 * (5 * w * 0.4167 + 24)
    return cost


def _balance_bounds(acts):
    def feasible(T):
        bounds = [0]
        for c in range(NCORES):
            nrem = NCORES - 1 - c
            lo = bounds[-1] + MINW
            hi = min(bounds[-1] + MAXW, W - nrem * MINW)
            lo = max(lo, W - nrem * MAXW)
            if lo > hi:
                return None
            best = lo
            for j in range(hi, lo - 1, -2):
                if _slab_cost(acts, bounds[-1], j) <= T:
                    best = j
                    break
            bounds.append(best if c < NCORES - 1 else W)
            if c == NCORES - 1 and _slab_cost(acts, bounds[-2], W) > T:
                return None
        if bounds[-1] != W:
            return None
        return bounds

    lo_T = 400_000.0
    hi_T = 2_000_000.0
    best = None
    for _ in range(14):
        mid = (lo_T + hi_T) / 2
        b = feasible(mid)
        if b is not None:
            best = b
            hi_T = mid
        else:
            lo_T = mid
    if best is None:
        best = list(range(0, W + 1, W // NCORES))
        best[-1] = W
    return best


def _bg_consts(w1, b1, w2, b2, w3, b3, conv_w, bn_gamma, bn_beta, bn_mean,
               bn_var):
    """Per-channel background value written by each layer k (fp64 host)."""
    g64 = np.float64
    inv = bn_gamma.astype(g64) / np.sqrt(bn_var.astype(g64) + BN_EPS)
    bnb = bn_beta.astype(g64) - bn_mean.astype(g64) * inv
    wsum = conv_w.astype(g64).sum(axis=(4, 5))   # [3, 2, C, C] (O, I)
    bg = np.zeros((6, C), g64)
    x_bg = np.zeros(C, g64)
    for blk in range(3):
        y = np.maximum(inv[blk, 0] * (wsum[blk, 0] @ x_bg) + bnb[blk, 0], 0)
        bg[2 * blk] = y
        x_bg = np.maximum(inv[blk, 1] * (wsum[blk, 1] @ y) + bnb[blk, 1]
                          + x_bg, 0)
        bg[2 * blk + 1] = x_bg
    return bg  # bg[k] = value this layer's dst holds on background cells


# ---------------------------------------------------------------------------
# program builder (one core)
# ---------------------------------------------------------------------------

def _build_program(plan, reps=1):
    import concourse.bacc as bacc
    import concourse.tile as tile
    from concourse import mybir
    from contextlib import ExitStack

    f32, f16, bf16 = mybir.dt.float32, mybir.dt.float16, mybir.dt.bfloat16
    Wc, WL = plan["W"], plan["WL"]
    w0 = plan["w0"]
    CELLS = HL * WL
    NT = (CELLS + RT_N - 1) // RT_N
    BSZ = DOFF + NT * RT_N + WL
    CXC = WL // 2            # buffer col c maps to centered cx = c - CXC
    # centered coordinate transforms (exact ints on the grid side)
    XMIN_C = XMIN + (w0 - HALO + CXC) * DX + 0.5 * DX
    YMIN_C = YMIN + 100 * DY + 0.5 * DY
    # gx' = (x - XMIN)/DX - (w0-HALO+CXC) - 0.5 ; cell cx' = (c%WL) - CXC - 0.5
    # shift both by +0.5 so grid coords are exact integers

    nc = bacc.Bacc("TRN2", target_bir_lowering=False, debug=False,
                   num_devices=1)

    d_pbox = nc.dram_tensor("pbox", [NBOX, 24], f32, kind="ExternalInput").ap()
    d_feat = nc.dram_tensor("featT26", [26, NBOX], f32, kind="ExternalInput").ap()
    d_w1b = nc.dram_tensor("w1b", [26, C], f32, kind="ExternalInput").ap()
    d_w2t = nc.dram_tensor("w2t", [128, 2 * C], f32, kind="ExternalInput").ap()
    d_w3t = nc.dram_tensor("w3t", [128, 2 * C], f32, kind="ExternalInput").ap()
    d_b1 = nc.dram_tensor("b1s", [128, 2], f32, kind="ExternalInput").ap()
    d_b2 = nc.dram_tensor("b2s", [128, 2], f32, kind="ExternalInput").ap()
    d_b3 = nc.dram_tensor("b3r", [1, C], f32, kind="ExternalInput").ap()
    d_sc = nc.dram_tensor("score", [NBOX, 1], f32, kind="ExternalInput").ap()
    d_eye = nc.dram_tensor("eye16", [128, 128], f32, kind="ExternalInput").ap()
    d_grid = nc.dram_tensor("grid", [10, NT * RT_N], bf16,
                            kind="ExternalInput").ap()
    d_cw = nc.dram_tensor("convw", [6, 128, 9 * 4 * 128], f16,
                          kind="ExternalInput").ap()
    d_bns = nc.dram_tensor("bnscale", [128, 12], f32, kind="ExternalInput").ap()
    d_bnb = nc.dram_tensor("bnbias", [128, 12], f32, kind="ExternalInput").ap()
    d_bgc = nc.dram_tensor("bgc", [128, 12], f16, kind="ExternalInput").ap()
    d_out = nc.dram_tensor("out", [C, H, Wc], f16, kind="ExternalOutput").ap()

    with tile.TileContext(nc) as tc:
        with ExitStack() as ctx:
            cpool = ctx.enter_context(tc.tile_pool(name="consts", bufs=1))

            bufs = [[cpool.tile([128, BSZ], f16, tag=f"buf{s}{cb}",
                                name=f"buf{s}{cb}")
                     for cb in range(2)] for s in range(2)]
            for s in range(2):
                for cb in range(2):
                    nc.vector.memset(bufs[s][cb][:], 0.0)

            t_feat = cpool.tile([26, NBOX], f32, tag="feat")
            nc.sync.dma_start(t_feat[:], d_feat)
            t_pbox = cpool.tile([NBOX, 24], f32, tag="pbox")
            nc.sync.dma_start(t_pbox[:], d_pbox)
            t_w1b = cpool.tile([26, C], f32, tag="w1b")
            nc.sync.dma_start(t_w1b[:], d_w1b)
            t_b1 = cpool.tile([128, 2], f32, tag="b1")
            nc.sync.dma_start(t_b1[:], d_b1)
            t_eye = cpool.tile([128, 128], f32, tag="eye")
            nc.sync.dma_start(t_eye[:], d_eye)
            t_w2t = cpool.tile([128, 2 * C], f32, tag="w2t")
            nc.sync.dma_start(t_w2t[:], d_w2t)
            t_w3t = cpool.tile([128, 2 * C], f32, tag="w3t")
            nc.sync.dma_start(t_w3t[:], d_w3t)
            t_b2 = cpool.tile([128, 2], f32, tag="b2")
            nc.sync.dma_start(t_b2[:], d_b2)
            t_b3 = cpool.tile([1, C], f32, tag="b3")
            nc.sync.dma_start(t_b3[:], d_b3)
            t_sc = cpool.tile([NBOX, 1], f32, tag="score")
            nc.sync.dma_start(t_sc[:], d_sc)
            t_bns = cpool.tile([128, 12], f32, tag="bns")
            nc.sync.dma_start(t_bns[:], d_bns)
            t_bnb = cpool.tile([128, 12], f32, tag="bnb")
            nc.sync.dma_start(t_bnb[:], d_bnb)
            t_bgc = cpool.tile([128, 12], f16, tag="bgc")
            nc.sync.dma_start(t_bgc[:], d_bgc)
            t_ones1 = cpool.tile([1, 128], f32, tag="ones1")
            nc.vector.memset(t_ones1[:], 1.0)
            t_ones16 = cpool.tile([128, 128], f16, tag="ones16")
            nc.vector.memset(t_ones16[:], 1.0)

            obj16 = cpool.tile([128, C], f16, tag="obj16")
            coefTall = cpool.tile([128, 128], bf16, tag="coefTall")

            # ---------------- MLP + box coefficients ----------------
            with ExitStack() as mctx:
                mpsum = mctx.enter_context(
                    tc.tile_pool(name="mpsum", bufs=2, space="PSUM"))
                msb = mctx.enter_context(tc.tile_pool(name="msb", bufs=2))

                h1 = msb.tile([128, 2 * 128], f32, tag="h1")
                for cb in range(2):
                    p = mpsum.tile([128, 128], f32, tag="mp")
                    nc.tensor.matmul(p[:], t_w1b[:, cb * 128:(cb + 1) * 128],
                                     t_feat[:], start=True, stop=True)
                    nc.scalar.activation(h1[:, cb * 128:(cb + 1) * 128], p[:],
                                         mybir.ActivationFunctionType.Relu,
                                         bias=t_b1[:, cb:cb + 1], scale=1.0)
                h2 = msb.tile([128, 2 * 128], f32, tag="h2")
                for cb in range(2):
                    p = mpsum.tile([128, 128], f32, tag="mp")
                    for b in range(2):
                        nc.tensor.matmul(
                            p[:],
                            t_w2t[:, b * C + cb * 128: b * C + (cb + 1) * 128],
                            h1[:, b * 128:(b + 1) * 128],
                            start=(b == 0), stop=(b == 1))
                    nc.scalar.activation(h2[:, cb * 128:(cb + 1) * 128], p[:],
                                         mybir.ActivationFunctionType.Relu,
                                         bias=t_b2[:, cb:cb + 1], scale=1.0)
                po = mpsum.tile([128, C], f32, tag="mpo")
                for b in range(2):
                    nc.tensor.matmul(po[:], h2[:, b * 128:(b + 1) * 128],
                                     t_w3t[:, b * C:(b + 1) * C],
                                     start=(b == 0), stop=False)
                nc.tensor.matmul(po[:], t_ones1[:], t_b3[:],
                                 start=False, stop=True)
                nc.vector.tensor_scalar_mul(obj16[:], po[:], t_sc[:])

                # centered gx/gy -> edge coefficients (f32), 3-way bf16 split
                g = msb.tile([128, 8], f32, tag="gxy")
                nc.vector.tensor_scalar(
                    g[:, 0:8:2], t_pbox[:, 0:12:3], -XMIN_C, 1.0 / DX,
                    mybir.AluOpType.add, mybir.AluOpType.mult)
                nc.vector.tensor_scalar(
                    g[:, 1:8:2], t_pbox[:, 1:12:3], -YMIN_C, 1.0 / DY,
                    mybir.AluOpType.add, mybir.AluOpType.mult)
                # coef[e] = (alpha=vx, beta=-vy, gamma=vy*ax - vx*ay)
                coefF = msb.tile([128, 12], f32, tag="coefF")
                tmp = msb.tile([128, 3], f32, tag="ctmp")
                for e in range(4):
                    en = (e + 1) % 4
                    nc.vector.tensor_tensor(
                        coefF[:, 3 * e:3 * e + 1], g[:, 2 * en:2 * en + 1],
                        g[:, 2 * e:2 * e + 1], mybir.AluOpType.subtract)
                    nc.vector.tensor_tensor(
                        tmp[:, 0:1], g[:, 2 * en + 1:2 * en + 2],
                        g[:, 2 * e + 1:2 * e + 2], mybir.AluOpType.subtract)
                    nc.vector.tensor_scalar_mul(
                        coefF[:, 3 * e + 1:3 * e + 2], tmp[:, 0:1], -1.0)
                    nc.vector.tensor_tensor(
                        tmp[:, 1:2], tmp[:, 0:1], g[:, 2 * e:2 * e + 1],
                        mybir.AluOpType.mult)
                    nc.vector.tensor_tensor(
                        tmp[:, 2:3], coefF[:, 3 * e:3 * e + 1],
                        g[:, 2 * e + 1:2 * e + 2], mybir.AluOpType.mult)
                    nc.vector.tensor_tensor(
                        coefF[:, 3 * e + 2:3 * e + 3], tmp[:, 1:2],
                        tmp[:, 2:3], mybir.AluOpType.subtract)
                # 3-way bf16 split: coefB16 cols per edge:
                # [ah am al bh bm bl gh gm gl -1]
                coefB16 = msb.tile([128, 40], bf16, tag="coefB16")
                nc.vector.memset(coefB16[:, 9:40:10], -1.0)
                rem = msb.tile([128, 12], f32, tag="rem")
                rem2 = msb.tile([128, 12], f32, tag="rem2")
                hi32 = msb.tile([128, 12], f32, tag="hi32")
                # lvl 0: hi = bf16(coef); rem = coef - hi
                for cc in range(3):
                    nc.vector.tensor_copy(coefB16[:, cc:40:10],
                                          coefF[:, cc:12:3])
                    nc.vector.tensor_copy(hi32[:, cc:12:3],
                                          coefB16[:, cc:40:10])
                nc.vector.tensor_tensor(rem[:], coefF[:], hi32[:],
                                        mybir.AluOpType.subtract)
                # lvl 1: mid = bf16(rem); rem2 = rem - mid
                for cc in range(3):
                    nc.vector.tensor_copy(coefB16[:, 3 + cc:40:10],
                                          rem[:, cc:12:3])
                    nc.vector.tensor_copy(hi32[:, cc:12:3],
                                          coefB16[:, 3 + cc:40:10])
                nc.vector.tensor_tensor(rem2[:], rem[:], hi32[:],
                                        mybir.AluOpType.subtract)
                # lvl 2: lo = bf16(rem2)
                for cc in range(3):
                    nc.vector.tensor_copy(coefB16[:, 6 + cc:40:10],
                                          rem2[:, cc:12:3])
                # transpose via f32 PE path (baseline-proven), downcast after
                coefB32 = msb.tile([128, 40], f32, tag="coefB32")
                nc.vector.tensor_copy(coefB32[:], coefB16[:])
                for e in range(4):
                    pt = mpsum.tile([10, 128], f32, tag="mptr")
                    nc.tensor.transpose(pt[:], coefB32[:, 10 * e:10 * e + 10],
                                        t_eye[:])
                    ct = msb.tile([10, 128], bf16, tag="ctT")
                    nc.vector.tensor_copy(ct[:], pt[:])
                    nc.sync.dma_start(coefTall[32 * e:32 * e + 10, :], ct[:])

            for _rep in range(reps):
                # ---------------- rasterization (dense) ----------------
                with ExitStack() as rctx:
                    gr_p = rctx.enter_context(tc.tile_pool(name="grid", bufs=3))
                    cr_p = rctx.enter_context(
                        tc.tile_pool(name="cross", bufs=4, space="PSUM"))
                    cnt_p = rctx.enter_context(
                        tc.tile_pool(name="cnt", bufs=1, space="PSUM"))
                    ft_p = rctx.enter_context(
                        tc.tile_pool(name="feat", bufs=2, space="PSUM"))
                    sc_p = rctx.enter_context(tc.tile_pool(name="rscr", bufs=2))
                    mk_p = rctx.enter_context(tc.tile_pool(name="mask", bufs=2))

                    for t in range(40):
                        for (rc0, rw) in plan["rspans"][t]:
                            n = 5 * rw
                            base = DOFF + (1 + 5 * t) * WL + rc0
                            cell0 = 5 * t * WL + rc0 + WL
                            gt = gr_p.tile([128, n], bf16, tag="g",
                                           padded_shape=[128, 512])
                            gsrc = d_grid[0:10, cell0:cell0 + 5 * WL]\
                                .rearrange("p (r c) -> p r c", r=5)[:, :, :rw]
                            for e in range(4):
                                nc.sync.dma_start(
                                    gt[32 * e:32 * e + 10, :], gsrc)
                            crs = []
                            for e in range(4):
                                cr = cr_p.tile([128, n], f32, tag="cr",
                                               padded_shape=[128, 512])
                                nc.tensor.matmul(
                                    cr[:], coefTall[32 * e:32 * e + 10, :],
                                    gt[32 * e:32 * e + 10, :],
                                    tile_position=(32 * e, 0),
                                    start=True, stop=True)
                                crs.append(cr)
                            s = sc_p.tile([128, n], f32, tag="mins",
                                          padded_shape=[128, 512])
                            nc.scalar.copy(s[:], crs[0][:])
                            for e in range(1, 4):
                                nc.vector.tensor_tensor(
                                    s[:], s[:], crs[e][:],
                                    mybir.AluOpType.min)
                            mask = mk_p.tile([128, n], f16, tag="m",
                                             padded_shape=[128, 512])
                            nc.vector.tensor_scalar(mask[:], s[:], 0.0, None,
                                                    mybir.AluOpType.is_ge)
                            cnt = cnt_p.tile([128, n], f32, tag="c",
                                             padded_shape=[128, 512])
                            nc.tensor.matmul(cnt[:], t_ones16[:], mask[:],
                                             start=True, stop=True)
                            rin = sc_p.tile([128, n], f32, tag="rin",
                                            padded_shape=[128, 512])
                            nc.vector.tensor_scalar_max(rin[:], cnt[:], 1.0)
                            r = sc_p.tile([128, n], f32, tag="r",
                                          padded_shape=[128, 512])
                            nc.vector.reciprocal_approx_fast(r[:], rin[:])
                            msc = mk_p.tile([128, n], f16, tag="msc",
                                            padded_shape=[128, 512])
                            nc.vector.tensor_tensor(msc[:], mask[:], r[:],
                                                    mybir.AluOpType.mult)
                            for cb in range(2):
                                ft = ft_p.tile([128, n], f32, tag="ft",
                                               padded_shape=[128, 512])
                                nc.tensor.matmul(
                                    ft[:], obj16[:, cb * 128:(cb + 1) * 128],
                                    msc[:], start=True, stop=True)
                                dst3 = bufs[0][cb][:, base:base + 5 * WL]\
                                    .rearrange("p (r c) -> p r c",
                                               r=5)[:, :, :rw]
                                ft3 = ft[:].rearrange("p (r c) -> p r c", r=5)
                                nc.scalar.copy(dst3, ft3)

                # ---------------- conv blocks (span-sparse) ----------------
                with ExitStack() as cctx:
                    w_p = cctx.enter_context(tc.tile_pool(name="cw", bufs=2))
                    cp_p = cctx.enter_context(
                        tc.tile_pool(name="cpsum", bufs=8, space="PSUM"))
                    st_p = cctx.enter_context(
                        tc.tile_pool(name="cstage", bufs=3))

                    for k in range(6):
                        j = k % 2
                        wk = w_p.tile([128, 9 * 4 * 128], f16, tag="wk")
                        nc.sync.dma_start(wk[:], d_cw[k])
                        src = bufs[k % 2]
                        dst = bufs[(k + 1) % 2]
                        kspans = plan["spans"][k]
                        kfills = plan["fills"][k]
                        # background fills of dst (complement regions)
                        for cb in range(2):
                            bgb = t_bgc[:, 2 * k + cb:2 * k + cb + 1]
                            for t in range(40):
                                for (c0, w) in kfills[t]:
                                    base = DOFF + (1 + 5 * t) * WL + c0
                                    d3 = dst[cb][:, base:base + 5 * WL]\
                                        .rearrange("p (r c) -> p r c",
                                                   r=5)[:, :, :w]
                                    bb = bgb.unsqueeze(1).to_broadcast(
                                        (128, 5, w))
                                    nc.vector.tensor_copy(d3, bb)
                        for t in range(40):
                            for (c0, w) in kspans[t]:
                                base = DOFF + (1 + 5 * t) * WL + c0
                                for cb in range(2):
                                    ps = cp_p.tile([128, 5 * w], f32, tag="ps",
                                                   padded_shape=[128, 512])
                                    ps3 = ps[:].rearrange("p (r c) -> p r c",
                                                          r=5)
                                    idx = 0
                                    for tap in range(9):
                                        dly, dlx = tap // 3 - 1, tap % 3 - 1
                                        delta = dly * WL + dlx
                                        for ci in range(2):
                                            lh = wk[:, ((tap * 2 + ci) * 2 + cb)
                                                    * 128:
                                                    ((tap * 2 + ci) * 2 + cb
                                                     + 1) * 128]
                                            rhs = src[ci][:, base + delta:
                                                          base + delta
                                                          + 5 * WL]
                                            rhs = rhs.rearrange(
                                                "p (r c) -> p r c",
                                                r=5)[:, :, :w]
                                            nc.tensor.matmul(
                                                ps[:], lh, rhs,
                                                start=(idx == 0),
                                                stop=(idx == 17))
                                            idx += 1
                                    sc_ap = t_bns[:, 2 * k + cb:2 * k + cb + 1]
                                    bi_ap = t_bnb[:, 2 * k + cb:2 * k + cb + 1]
                                    dsl = dst[cb][:, base:base + 5 * WL]\
                                        .rearrange("p (r c) -> p r c",
                                                   r=5)[:, :, :w]
                                    if j == 0:
                                        nc.scalar.activation(
                                            dsl, ps3,
                                            mybir.ActivationFunctionType.Relu,
                                            bias=bi_ap, scale=sc_ap)
                                    else:
                                        bn = st_p.tile([128, 5 * w], f32,
                                                       tag="bn",
                                                       padded_shape=[128, 512])
                                        bn3 = bn[:].rearrange(
                                            "p (r c) -> p r c", r=5)
                                        nc.scalar.activation(
                                            bn3, ps3,
                                            mybir.ActivationFunctionType
                                            .Identity,
                                            bias=bi_ap, scale=sc_ap)
                                        nc.vector.tensor_tensor(
                                            dsl, bn3, dsl,
                                            mybir.AluOpType.add)
                                        nc.vector.tensor_scalar_max(
                                            dsl, dsl, 0.0)

                # ---------------- output DMA ----------------
                for cb in range(2):
                    for rch in range(8):
                        r0 = rch * 25
                        base = DOFF + (1 + r0) * WL + HALO
                        s3 = bufs[0][cb][:, base:base + 25 * WL].rearrange(
                            "p (r c) -> p r c", r=25)[:, :, :Wc]
                        nc.sync.dma_start(
                            d_out[cb * 128:(cb + 1) * 128, r0:r0 + 25, :],
                            s3)
    nc.compile()
    return nc


# ---------------------------------------------------------------------------
# host-side input prep
# ---------------------------------------------------------------------------

def _prep_shared(pred_box, pred_score, w1, b1, w2, b2, w3, b3,
                 conv_w, bn_gamma, bn_beta, bn_mean, bn_var):
    import ml_dtypes
    f32 = np.float32
    pbox = np.ascontiguousarray(pred_box.reshape(NBOX, 24).astype(f32))
    feat = np.concatenate([pbox, pred_score.reshape(NBOX, 1).astype(f32)],
                          axis=1)
    featT26 = np.concatenate(
        [feat.T, np.ones((1, NBOX), f32)], axis=0).astype(f32)
    w1b = np.concatenate([w1.astype(f32), b1.reshape(1, C).astype(f32)],
                         axis=0)

    def two_blk(w):
        n = w.shape[1]
        o = np.empty((128, 2 * n), f32)
        o[:, :n] = w[:128]
        o[:, n:] = w[128:]
        return np.ascontiguousarray(o)

    w2t = two_blk(w2.astype(f32))
    w3t = two_blk(w3.astype(f32))
    b1s = np.ascontiguousarray(b1.astype(f32).reshape(2, 128).T)
    b2s = np.ascontiguousarray(b2.astype(f32).reshape(2, 128).T)
    b3r = b3.astype(f32).reshape(1, C)
    score = np.ascontiguousarray(pred_score.astype(f32).reshape(NBOX, 1))
    eye16 = np.eye(128, dtype=f32)

    cw = conv_w.astype(f32).reshape(6, C, C, 3, 3)
    cwt = cw.transpose(0, 3, 4, 2, 1)
    cwt = cwt.reshape(6, 9, 2, 128, 2, 128)
    cwt = cwt.transpose(0, 3, 1, 2, 4, 5)
    convw = np.ascontiguousarray(
        cwt.reshape(6, 128, 9 * 4 * 128).astype(np.float16))

    g64 = np.float64
    inv = (bn_gamma.astype(g64) / np.sqrt(bn_var.astype(g64) + BN_EPS))
    bnb = (bn_beta.astype(g64) - bn_mean.astype(g64) * inv)
    bns_ = np.empty((128, 12), f32)
    bnb_ = np.empty((128, 12), f32)
    for k in range(6):
        for cb in range(2):
            bns_[:, 2 * k + cb] = inv.reshape(6, C)[k][cb * 128:(cb + 1) * 128]
            bnb_[:, 2 * k + cb] = bnb.reshape(6, C)[k][cb * 128:(cb + 1) * 128]

    bg = _bg_consts(w1, b1, w2, b2, w3, b3, conv_w, bn_gamma, bn_beta,
                    bn_mean, bn_var)
    bgc = np.empty((128, 12), np.float16)
    for k in range(6):
        for cb in range(2):
            bgc[:, 2 * k + cb] = bg[k][cb * 128:(cb + 1) * 128]

    return dict(pbox=pbox, featT26=featT26, w1b=w1b, w2t=w2t, w3t=w3t,
                b1s=b1s, b2s=b2s, b3r=b3r, score=score, eye16=eye16,
                convw=convw, bnscale=bns_, bnbias=bnb_, bgc=bgc)


def _core_grid(plan):
    import ml_dtypes
    WL = plan["WL"]
    CELLS = HL * WL
    NT = (CELLS + RT_N - 1) // RT_N
    CXC = WL // 2
    n = NT * RT_N
    cell = np.arange(n)
    hh = cell // WL - 1
    ww_l = cell % WL
    cy = (hh - 100).astype(np.float32)           # exact ints
    cx = (ww_l - CXC).astype(np.float32)
    ww = plan["w0"] - HALO + ww_l
    inval = (((hh < 0) | (hh >= H) | (ww < 0) | (ww >= W)
              | (cell >= CELLS)).astype(np.float32) * 1e9)
    one = np.ones(n, np.float32)
    # rows match the coef column layout [ah bh gh am bm gm al bl gl -1]
    g = np.stack([cy, cx, one, cy, cx, one, cy, cx, one, inval])
    return np.ascontiguousarray(g.astype(ml_dtypes.bfloat16))


def _prep_inputs(pred_box, pred_score, w1, b1, w2, b2, w3, b3,
                 conv_w, bn_gamma, bn_beta, bn_mean, bn_var):
    plans = _make_plans(np.asarray(pred_box))
    shared = _prep_shared(pred_box, pred_score, w1, b1, w2, b2, w3, b3,
                          conv_w, bn_gamma, bn_beta, bn_mean, bn_var)
    in_maps = [dict(shared, grid=_core_grid(p)) for p in plans]
    return plans, in_maps


# ---------------------------------------------------------------------------
# per-core concurrent dispatch
# ---------------------------------------------------------------------------

def _make_core_fn(nc, device):
    """Compiled single-device callable for one core's program."""
    import jax
    from concourse import mybir
    from concourse.bass2jax import (_bass_exec_p, install_neuronx_cc_hook,
                                    partition_id_tensor)

    install_neuronx_cc_hook()
    partition_name = (nc.partition_id_tensor.name
                      if nc.partition_id_tensor else None)
    in_names, out_names, out_avals, zero_outs = [], [], [], []
    for alloc in nc.m.functions[0].allocations:
        if not isinstance(alloc, mybir.MemoryLocationSet):
            continue
        name = alloc.memorylocations[0].name
        if alloc.kind == "ExternalInput":
            if name != partition_name:
                in_names.append(name)
        elif alloc.kind == "ExternalOutput":
            shape = tuple(alloc.tensor_shape)
            dtype = mybir.dt.np(alloc.dtype)
            out_names.append(name)
            out_avals.append(jax.core.ShapedArray(shape, dtype))
            zero_outs.append(np.zeros(shape, dtype))
    n_params = len(in_names)
    bind_in_names = list(in_names) + list(out_names)
    if partition_name is not None:
        bind_in_names.append(partition_name)

    def _body(*args):
        operands = list(args)
        if partition_name is not None:
            operands.append(partition_id_tensor())
        outs = _bass_exec_p.bind(
            *operands,
            out_avals=tuple(out_avals),
            in_names=tuple(bind_in_names),
            out_names=tuple(out_names),
            lowering_input_output_aliases=(),
            sim_require_finite=True,
            sim_require_nnan=True,
            nc=nc,
        )
        return tuple(outs)

    fn = jax.jit(_body, keep_unused=True)
    return fn, in_names, out_names, zero_outs, device


def _dispatch_all(core_fns, in_maps):
    """Dispatch all cores async, then block; returns per-core out dicts."""
    import jax
    futs = []
    for (fn, in_names, out_names, zero_outs, device), im in zip(core_fns,
                                                                in_maps):
        args = [jax.device_put(np.asarray(im[n]), device) for n in in_names]
        args += [jax.device_put(z, device) for z in zero_outs]
        futs.append((fn(*args), out_names))
    jax.block_until_ready([f for f, _ in futs])
    return [{n: np.asarray(o) for n, o in zip(names, outs)}
            for outs, names in futs]


_CACHED = {}


def _get_programs(pred_box, reps=1):
    import jax
    key = (np.asarray(pred_box, np.float32).tobytes(), reps)
    if key not in _CACHED:
        plans = _make_plans(np.asarray(pred_box))
        devices = jax.devices()[:NCORES]
        ncs = [_build_program(p, reps=reps) for p in plans]
        core_fns = [_make_core_fn(nc, d) for nc, d in zip(ncs, devices)]
        _CACHED[key] = (plans, ncs, core_fns)
    return _CACHED[key]


def kernel(**inputs) -> np.ndarray:
    inputs = {k: np.asarray(v) for k, v in inputs.items()}
    plans, ncs, core_fns = _get_programs(inputs["pred_box"])
    _, in_maps = _prep_inputs(**inputs)
    res = _dispatch_all(core_fns, in_maps)
    out = np.empty((C, H, W), np.float32)
    for core, p in enumerate(plans):
        out[:, :, p["w0"]:p["w0"] + p["W"]] = res[core]["out"].astype(
            np.float32)
    return out


if __name__ == "__main__":
    import reference as R

    inp = {k: np.asarray(v) for k, v in R.setup_inputs().items()}
    got = kernel(**inp)
    exp = np.asarray(R.reference(**inp))
    err = np.abs(got - exp)
    rel = np.linalg.norm(got - exp) / np.linalg.norm(exp)
    print("absmax err:", err.max(), " absmax ref:", np.abs(exp).max())
    print("Relative error:", rel)



# revision 26
# speedup vs baseline: 24.0233x; 24.0233x over previous
"""Trainium2 Bass kernel for Box2FeatureGeneratorV2 — sparse span edition.

Key ideas vs the dense baseline:
  1. feat_sum is EXACTLY zero outside the rasterized boxes, so away from
     boxes every layer's activation equals a per-channel constant
     ("background"), computable on the host from the weights alone.
     Each conv layer therefore only computes spans covering
     (box support dilated by k+1) plus the H/W border frames; everything
     else is filled with the background constant (exact, not approximate).
  2. The span schedule is data-dependent, so kernel() compiles one
     specialized program PER CORE per box layout (cached), and dispatches
     the 8 single-core programs concurrently on the 8 NeuronCores.
  3. Raster edge tests use exact 3-way bf16 splits of the edge
     coefficients against integer-centered cell coords (exact in bf16):
     K=10 bf16 matmuls run at 1 cycle/row instead of fp32's 4.
  4. Slab widths are load-balanced per call from the span plan.
"""

import sys
import numpy as np

sys.path.insert(0, "/opt/trn_rl_repo")

H, W, C, NBOX = 200, 704, 256, 128
NCORES = 8
HALO = 6
HL = H + 2                  # 202 buffer rows (1 zero row each side)
DOFF = 4
RT_N = 505                  # raster tile free size
XMIN, YMIN, DX, DY = -140.8, -40.0, 0.4, 0.4
BN_EPS = 1e-5
MAXW = 96                   # max slab width (SBUF limit)
MINW = 40
GAP = 4                     # merge spans with gaps <= GAP
SPAN_MAX = 102              # max span width (PSUM bank: 5*102=510 fp32)


# ---------------------------------------------------------------------------
# host-side planning
# ---------------------------------------------------------------------------

def _inside_mask(pred_box):
    """[H, W] bool: cells inside any box (mirrors reference fp32 math)."""
    f32 = np.float32
    gx = ((pred_box[:, :4, 0] - XMIN) / DX).astype(f32)
    gy = ((pred_box[:, :4, 1] - YMIN) / DY).astype(f32)
    cxs = (np.arange(W, dtype=f32) + 0.5)
    cys = (np.arange(H, dtype=f32) + 0.5)
    inside = np.ones((NBOX, H, W), bool)
    for e in range(4):
        ax, ay = gx[:, e], gy[:, e]
        bx, by = gx[:, (e + 1) % 4], gy[:, (e + 1) % 4]
        vx, vy = bx - ax, by - ay
        c = (vx[:, None, None] * (cys[None, :, None] - ay[:, None, None])
             - vy[:, None, None] * (cxs[None, None, :] - ax[:, None, None]))
        inside &= (c >= 0)
    return inside.any(0)


def _dilate(m):
    out = m.copy()
    out[:-1] |= m[1:]
    out[1:] |= m[:-1]
    out2 = out.copy()
    out2[:, :-1] |= out[:, 1:]
    out2[:, 1:] |= out[:, :-1]
    return out2


def _runs(colact, gap, max_w):
    runs = []
    in_run = False
    start = 0
    for c in range(len(colact)):
        if colact[c] and not in_run:
            start = c
            in_run = True
        elif not colact[c] and in_run:
            runs.append([start, c])
            in_run = False
    if in_run:
        runs.append([start, len(colact)])
    merged = []
    for s, e in runs:
        if merged and s - merged[-1][1] <= gap:
            merged[-1][1] = e
        else:
            merged.append([s, e])
    out = []
    for s, e in merged:
        w = e - s
        nsub = (w + max_w - 1) // max_w
        for j in range(nsub):
            s0 = s + j * max_w
            out.append((s0, min(max_w, e - s0)))
    return out


def _mask_runs_per_tile(mask, gap, max_w):
    """mask: [H, WL] bool -> per-5-row-tile col runs."""
    out = []
    for t in range(40):
        out.append(_runs(mask[5 * t:5 * t + 5].any(0), gap, max_w))
    return out


def _rect_cover(mask):
    """Greedy rectangle cover of a [H, WL] bool mask: for each maximal
    band of rows with identical column patterns, emit (r0, nr, c0, nc)
    rects per column run.  Good enough for border frames (few bands)."""
    rects = []
    r = 0
    Hh = mask.shape[0]
    while r < Hh:
        if not mask[r].any():
            r += 1
            continue
        r1 = r + 1
        while r1 < Hh and (mask[r1] == mask[r]).all():
            r1 += 1
        row = mask[r]
        c = 0
        WLl = mask.shape[1]
        while c < WLl:
            if not row[c]:
                c += 1
                continue
            c1 = c + 1
            while c1 < WLl and row[c1]:
                c1 += 1
            rects.append((r, r1 - r, c, c1 - c))
            c = c1
        r = r1
    return rects


def _make_plans(pred_box):
    support = _inside_mask(pred_box)
    occ = _dilate(support)      # +1 safety margin for the raster spans
    acts = []                   # act[k] = dilate^{k+1}(support), no borders
    a = support
    for k in range(6):
        a = _dilate(a)
        acts.append(a.copy())

    bounds = _balance_bounds(acts)

    plans = []
    for core in range(NCORES):
        w0, w1 = bounds[core], bounds[core + 1]
        Wc = w1 - w0
        WL = Wc + 2 * HALO
        lo_map = w0 - HALO          # map col of buffer col 0

        def loc(mask_map):
            """[H, W] map mask -> [H, WL] buffer-local mask."""
            m = np.zeros((H, WL), bool)
            a0 = max(0, -lo_map)
            b0 = max(0, lo_map)
            n = min(WL, W - lo_map) - a0
            if n > 0:
                m[:, a0:a0 + n] = mask_map[:, b0:b0 + n]
            return m

        def frame_mask(d):
            """cells within dist d of a MAP edge, in buffer-local coords,
            clipped to in-map cells."""
            m = np.zeros((H, WL), bool)
            cols = np.arange(WL) + lo_map
            inmap = (cols >= 0) & (cols < W)
            m[:d, inmap] = True
            m[H - d:, inmap] = True
            m[:, inmap & (cols < d)] = True
            m[:, inmap & (cols >= W - d)] = True
            return m

        acts_l = [loc(acts[k]) for k in range(6)]
        occ_l = loc(occ)
        windows = []
        for k in range(6):
            ilo = max(k + 1, -lo_map)
            ihi = min(WL - (k + 1), W - lo_map)
            windows.append((ilo, ihi))

        spans = []           # [k][t] -> list[(c0,w)]
        runsmask = []        # [k]: [H, WL] cells WRITTEN by layer k's spans
        for k in range(6):
            ilo, ihi = windows[k]
            am = acts_l[k].copy()
            am[:, :ilo] = False
            am[:, ihi:] = False
            sp = _mask_runs_per_tile(am, GAP, SPAN_MAX)
            spans.append(sp)
            rm = np.zeros((H, WL), bool)
            for t in range(40):
                for (c0, w) in sp[t]:
                    rm[5 * t:5 * t + 5, c0:c0 + w] = True
            runsmask.append(rm)

        def dil(m):
            return _dilate(m)

        readmask = [dil(runsmask[k]) for k in range(6)]

        # raster spans over occ
        ilo = max(0, -lo_map)
        ihi = min(WL, W - lo_map)
        om = occ_l.copy()
        om[:, :ilo] = False
        om[:, ihi:] = False
        rspans = _mask_runs_per_tile(om, 6, SPAN_MAX)
        rspm = np.zeros((H, WL), bool)
        for t in range(40):
            for (c0, w) in rspans[t]:
                rspm[5 * t:5 * t + 5, c0:c0 + w] = True
        # packed-grid offsets, in rspan scan order
        goffs = []
        gtot = 0
        for t in range(40):
            row = []
            for (c0, w) in rspans[t]:
                row.append(gtot)
                gtot += 5 * w
            goffs.append(row)

        # ---- background fills per layer (value bg_k, dst of layer k) ----
        # Must cover every cell the buffer serves before its next
        # overwrite: conv reads of layer k+1 (readmask) plus, for odd k,
        # the residual reads of layer k+2 (runsmask[k+2]).  Excludes the
        # span runs themselves and frame cells (those hold border-field
        # values from the border DMAs).
        fillm = []
        for k in range(6):
            if k < 5:
                m = readmask[k + 1].copy()
                if k + 2 <= 5 and k % 2 == 1:
                    m |= runsmask[k + 2]
                m &= ~runsmask[k] & ~frame_mask(k + 1)
            else:
                m = np.zeros((H, WL), bool)
            fillm.append(m)

        out_mask = np.zeros((H, WL), bool)
        out_mask[:, HALO:WL - HALO] = True

        # fills as 5-row-tile col-runs (gap-merged), rows clipped out of
        # the horizontal frame bands (frame cells hold border-field DMAs).
        # Cells inside an emitted rect but outside fillm are either span
        # cells (recomputed right after, same layer) or background cells
        # for which bg_k is the correct value.
        fills = [None] * 6
        fillemit = [None] * 6
        for k in range(5):
            d = k + 1
            m = fillm[k]
            rects = []
            em = np.zeros((H, WL), bool)
            for t in range(40):
                r_lo, r_hi = 5 * t, 5 * t + 5
                sub = m[r_lo:r_hi]
                if not sub.any():
                    continue
                rr_lo = max(r_lo, d)
                rr_hi = min(r_hi, H - d)
                for (c0, w) in _runs(sub.any(0), 3, 10 ** 9):
                    rects.append((rr_lo, rr_hi - rr_lo, c0, w))
                    em[rr_lo:rr_hi, c0:c0 + w] = True
            fills[k] = rects
            fillemit[k] = em

        # ---- border frames (host field canvas values) ----
        needed = []
        for k in range(6):
            fm = frame_mask(k + 1)
            if k < 5:
                rd = readmask[k + 1].copy()
                if k % 2 == 1 and k + 2 <= 5:
                    rd |= runsmask[k + 2]   # residual reads of layer k+2
                nk = rd & fm & ~runsmask[k]
            else:
                nk = out_mask & fm & ~runsmask[5]
            needed.append(nk)

        # startup bg5 fill: output cells never written in-body
        inbody_writes_b0 = (rspm | runsmask[1] | runsmask[3] | runsmask[5]
                            | fillemit[1] | fillemit[3])
        bg5m = out_mask & ~runsmask[5] & ~needed[5] & ~inbody_writes_b0
        bg5fill = _rect_cover(bg5m)
        # leftovers dirtied in-body: refill with bg5 at layer-5 fill time
        left5 = (out_mask & ~runsmask[5] & ~needed[5] & inbody_writes_b0)
        fills[5] = [(r0, nr, c0, nc)
                    for (r0, nr, c0, nc) in _rect_cover(left5)]
        fillemit[5] = left5

        # classify border rects: in-body at layer k if any same-buffer
        # write that runs before layer k+1's reads overlaps (fills/spans/
        # raster of earlier layers), or a later startup border would
        # clobber the value.
        bord_startup = []
        bord_inbody = []
        for k in range(6):
            if not needed[k].any():
                continue
            kb = (k + 1) % 2
            clob = np.zeros((H, WL), bool)
            for k2 in range(k + 1, 6):
                if (k2 + 1) % 2 == kb:
                    clob |= needed[k2]
            for j in range(k):
                if (j + 1) % 2 == kb:
                    clob |= runsmask[j] | fillemit[j]
            if kb == 0:
                clob |= rspm
            for (r0, nr, c0, nc) in _rect_cover(needed[k]):
                if clob[r0:r0 + nr, c0:c0 + nc].any():
                    bord_inbody.append((k, r0, nr, c0, nc))
                else:
                    bord_startup.append((k, r0, nr, c0, nc))

        plans.append(dict(w0=w0, W=Wc, WL=WL, spans=spans, fills=fills,
                          rspans=rspans, goffs=goffs, gtot=gtot,
                          bord_startup=bord_startup, bord_inbody=bord_inbody,
                          bg5fill=bg5fill))
    return plans


def _slab_cost(acts, w0, w1):
    """Analytic per-core body cost (ns) for slab [w0, w1), calibrated
    against measured per-core bodies of the v1 kernel (0.51 ns/col
    marginal, ~31 ns matmul floor, ~0.4 us per-span fixed)."""
    Wc = w1 - w0
    WL = Wc + 2 * HALO
    lo_map = w0 - HALO
    cost = 30000.0
    for k in range(6):
        ab = acts[k]
        vlo, vhi = k + 1, WL - (k + 1)
        ilo = max(vlo, -lo_map)
        ihi = min(vhi, W - lo_map)
        for t in range(40):
            sl = ab[5 * t:5 * t + 5, max(0, lo_map):min(W, lo_map + WL)]
            colact = np.zeros(WL, bool)
            colact[max(0, -lo_map):max(0, -lo_map) + sl.shape[1]] = sl.any(0)
            colact[:ilo] = False
            colact[ihi:] = False
            for s, w in _runs(colact, GAP, SPAN_MAX):
                cost += 36 * max(31.0, 0.51 * 5 * w) + 400.0
    occ0 = acts[0]
    ilo = max(0, -lo_map)
    ihi = min(WL, W - lo_map)
    for t in range(40):
        sl = occ0[5 * t:5 * t + 5, max(0, lo_map):min(W, lo_map + WL)]
        colact = np.zeros(WL, bool)
        colact[max(0, -lo_map):max(0, -lo_map) + sl.shape[1]] = sl.any(0)
        colact[:ilo] = False
        colact[ihi:] = False
        for s, w in _runs(colact, 6, SPAN_MAX):
            cost += 7 * max(31.0, 0.51 * 5 * w) + 600.0
    return cost


def _balance_bounds(acts):
    # per-(k, tile) column-activity prefix sums over the full map
    pref = np.zeros((6, 40, W + 1), np.int32)
    anyp = np.zeros((6, 40, W + 1), np.int32)
    for k in range(6):
        for t in range(40):
            ca = acts[k][5 * t:5 * t + 5].any(0)
            pref[k, t, 1:] = np.cumsum(ca)
            anyp[k, t, 1:] = np.cumsum(ca)  # same; kept for clarity
    occ = _dilate(acts[0]) if False else acts[0]
    rpref = np.zeros((40, W + 1), np.int32)
    for t in range(40):
        rpref[t, 1:] = np.cumsum(occ[5 * t:5 * t + 5].any(0))

    def cost(w0, w1):
        c = 30000.0
        for k in range(6):
            ext = HALO - (k + 1)
            a = max(0, w0 - ext)
            b = min(W, w1 + ext)
            cnt = pref[k, :, b] - pref[k, :, a]
            cells = 5 * int(cnt.sum())
            ntile = int((cnt > 0).sum())
            c += 18.4 * cells + 560.0 * ntile
        cnt = rpref[:, min(W, w1 + HALO)] - rpref[:, max(0, w0 - HALO)]
        c += 1.8 * 5 * int(cnt.sum()) + 700.0 * int((cnt > 0).sum())
        return c

    def feasible(T):
        bounds = [0]
        for c_i in range(NCORES):
            nrem = NCORES - 1 - c_i
            if nrem == 0:
                if cost(bounds[-1], W) > T:
                    return None
                bounds.append(W)
                break
            lo = max(bounds[-1] + MINW, W - (bounds[-1] + 0) - 0, 0)
            lo = max(bounds[-1] + MINW, W - nrem * MAXW)
            hi = min(bounds[-1] + MAXW, W - nrem * MINW)
            if lo > hi:
                return None
            best = None
            for j in range(hi, lo - 1, -1):
                if cost(bounds[-1], j) <= T:
                    best = j
                    break
            if best is None:
                return None
            bounds.append(best)
        return bounds

    lo_T, hi_T = 100_000.0, 3_000_000.0
    best = None
    for _ in range(22):
        mid = (lo_T + hi_T) / 2
        b = feasible(mid)
        if b is not None:
            best = b
            hi_T = mid
        else:
            lo_T = mid
    if best is None:
        best = list(range(0, W + 1, W // NCORES))
        best[-1] = W
    return best


def _bg_consts(w1, b1, w2, b2, w3, b3, conv_w, bn_gamma, bn_beta, bn_mean,
               bn_var):
    """Per-channel background value written by each layer k (fp64 host)."""
    g64 = np.float64
    inv = bn_gamma.astype(g64) / np.sqrt(bn_var.astype(g64) + BN_EPS)
    bnb = bn_beta.astype(g64) - bn_mean.astype(g64) * inv
    wsum = conv_w.astype(g64).sum(axis=(4, 5))   # [3, 2, C, C] (O, I)
    bg = np.zeros((6, C), g64)
    x_bg = np.zeros(C, g64)
    for blk in range(3):
        y = np.maximum(inv[blk, 0] * (wsum[blk, 0] @ x_bg) + bnb[blk, 0], 0)
        bg[2 * blk] = y
        x_bg = np.maximum(inv[blk, 1] * (wsum[blk, 1] @ y) + bnb[blk, 1]
                          + x_bg, 0)
        bg[2 * blk + 1] = x_bg
    return bg  # bg[k] = value this layer's dst holds on background cells


# ---------------------------------------------------------------------------
# program builder (one core)
# ---------------------------------------------------------------------------

def _bord_layout(plan):
    """Deterministic (host+builder shared) layout of the border tensor:
    returns (entries, total) with entries = [(k, r0, nr, c0, nc, cb, off,
    inbody)]; off indexes the per-cb half of d_bord."""
    entries = []
    off = 0
    for inbody, lst in ((False, plan["bord_startup"]),
                        (True, plan["bord_inbody"])):
        for (k, r0, nr, c0, nc) in lst:
            for cb in range(2):
                entries.append((k, r0, nr, c0, nc, cb, off, inbody))
            off += nr * nc
    return entries, off


def _build_program(plan, reps=1):
    import concourse.bacc as bacc
    import concourse.tile as tile
    from concourse import mybir
    from contextlib import ExitStack

    f32, f16, bf16 = mybir.dt.float32, mybir.dt.float16, mybir.dt.bfloat16
    Wc, WL = plan["W"], plan["WL"]
    w0 = plan["w0"]
    BSZ = DOFF + (HL + 1) * WL
    GTOT = max(plan["gtot"], 4)
    bord_entries, BTOT = _bord_layout(plan)
    BTOT = max(BTOT, 4)
    CXC = WL // 2            # buffer col c maps to centered cx = c - CXC
    # centered coordinate transforms (exact ints on the grid side)
    XMIN_C = XMIN + (w0 - HALO + CXC) * DX + 0.5 * DX
    YMIN_C = YMIN + 100 * DY + 0.5 * DY
    # gx' = (x - XMIN)/DX - (w0-HALO+CXC) - 0.5 ; cell cx' = (c%WL) - CXC - 0.5
    # shift both by +0.5 so grid coords are exact integers

    nc = bacc.Bacc("TRN2", target_bir_lowering=False, debug=False,
                   num_devices=1)

    d_pbox = nc.dram_tensor("pbox", [NBOX, 24], f32, kind="ExternalInput").ap()
    d_feat = nc.dram_tensor("featT26", [26, NBOX], f32, kind="ExternalInput").ap()
    d_w1b = nc.dram_tensor("w1b", [26, C], f32, kind="ExternalInput").ap()
    d_w2t = nc.dram_tensor("w2t", [128, 2 * C], f32, kind="ExternalInput").ap()
    d_w3t = nc.dram_tensor("w3t", [128, 2 * C], f32, kind="ExternalInput").ap()
    d_b1 = nc.dram_tensor("b1s", [128, 2], f32, kind="ExternalInput").ap()
    d_b2 = nc.dram_tensor("b2s", [128, 2], f32, kind="ExternalInput").ap()
    d_b3 = nc.dram_tensor("b3r", [1, C], f32, kind="ExternalInput").ap()
    d_sc = nc.dram_tensor("score", [NBOX, 1], f32, kind="ExternalInput").ap()
    d_eye = nc.dram_tensor("eye16", [128, 128], f32, kind="ExternalInput").ap()
    d_gridp = nc.dram_tensor("gridp", [128, GTOT], bf16,
                             kind="ExternalInput").ap()
    d_bord = nc.dram_tensor("bord", [128, 2 * BTOT], f16,
                            kind="ExternalInput").ap()
    d_cw = nc.dram_tensor("convw", [6, 128, 9 * 4 * 128], f16,
                          kind="ExternalInput").ap()
    d_bns = nc.dram_tensor("bnscale", [128, 12], f32, kind="ExternalInput").ap()
    d_bnb = nc.dram_tensor("bnbias", [128, 12], f32, kind="ExternalInput").ap()
    d_bgc = nc.dram_tensor("bgc", [128, 12], f16, kind="ExternalInput").ap()
    d_out = nc.dram_tensor("out", [C, H, Wc], f16, kind="ExternalOutput").ap()

    with tile.TileContext(nc) as tc:
        with ExitStack() as ctx:
            cpool = ctx.enter_context(tc.tile_pool(name="consts", bufs=1))

            bufs = [[cpool.tile([128, BSZ], f16, tag=f"buf{s}{cb}",
                                name=f"buf{s}{cb}")
                     for cb in range(2)] for s in range(2)]
            for s in range(2):
                for cb in range(2):
                    nc.vector.memset(bufs[s][cb][:], 0.0)

            t_bns = cpool.tile([128, 12], f32, tag="bns")
            nc.sync.dma_start(t_bns[:], d_bns)
            t_bnb = cpool.tile([128, 12], f32, tag="bnb")
            nc.sync.dma_start(t_bnb[:], d_bnb)
            t_bgc = cpool.tile([128, 12], f16, tag="bgc")
            nc.sync.dma_start(t_bgc[:], d_bgc)
            t_ones1 = cpool.tile([1, 128], f32, tag="ones1")
            nc.vector.memset(t_ones1[:], 1.0)
            t_ones16 = cpool.tile([128, 128], f16, tag="ones16")
            nc.vector.memset(t_ones16[:], 1.0)

            # SBUF-resident packed raster grid: one startup DMA replaces
            # the per-rspan grid DMAs that saturated the SP sequencer.
            t_gridp = cpool.tile([128, GTOT], bf16, tag="gridp")
            nc.sync.dma_start(t_gridp[:], d_gridp)

            obj16 = cpool.tile([128, C], f16, tag="obj16")
            coefTall = cpool.tile([128, 128], bf16, tag="coefTall")

            # ---- startup background + border writes (never dirtied
            # in-body: complement of act_5 and frame cells are written
            # only here) ----
            for cb in range(2):
                bgb = t_bgc[:, 10 + cb:10 + cb + 1]
                for (r0, nr, c0, nc_) in plan["bg5fill"]:
                    base = DOFF + (1 + r0) * WL + c0
                    d3 = bufs[0][cb][:, base:base + nr * WL]\
                        .rearrange("p (r c) -> p r c", r=nr)[:, :, :nc_]
                    bb = bgb.unsqueeze(1).to_broadcast((128, nr, nc_))
                    nc.vector.tensor_copy(d3, bb)
            for (k, r0, nr, c0, nc_, cb, off, inbody) in bord_entries:
                if inbody:
                    continue
                dstb = bufs[(k + 1) % 2][cb]
                base = DOFF + (1 + r0) * WL + c0
                d3 = dstb[:, base:base + nr * WL]\
                    .rearrange("p (r c) -> p r c", r=nr)[:, :, :nc_]
                s3 = d_bord[:, cb * BTOT + off: cb * BTOT + off + nr * nc_]\
                    .rearrange("p (r c) -> p r c", r=nr)
                nc.sync.dma_start(d3, s3)

            # ---------------- MLP + box coefficients ----------------
            with ExitStack() as mctx:
                mpsum = mctx.enter_context(
                    tc.tile_pool(name="mpsum", bufs=2, space="PSUM"))
                msb = mctx.enter_context(tc.tile_pool(name="msb", bufs=2))
                mcst = mctx.enter_context(tc.tile_pool(name="mconsts",
                                                       bufs=1))

                # MLP-only constants live in an MLP-scoped pool so their
                # SBUF is released before the conv section allocates.
                t_feat = mcst.tile([26, NBOX], f32, tag="feat")
                nc.sync.dma_start(t_feat[:], d_feat)
                t_pbox = mcst.tile([NBOX, 24], f32, tag="pbox")
                nc.sync.dma_start(t_pbox[:], d_pbox)
                t_w1b = mcst.tile([26, C], f32, tag="w1b")
                nc.sync.dma_start(t_w1b[:], d_w1b)
                t_b1 = mcst.tile([128, 2], f32, tag="b1")
                nc.sync.dma_start(t_b1[:], d_b1)
                t_eye = mcst.tile([128, 128], f32, tag="eye")
                nc.sync.dma_start(t_eye[:], d_eye)
                t_w2t = mcst.tile([128, 2 * C], f32, tag="w2t")
                nc.sync.dma_start(t_w2t[:], d_w2t)
                t_w3t = mcst.tile([128, 2 * C], f32, tag="w3t")
                nc.sync.dma_start(t_w3t[:], d_w3t)
                t_b2 = mcst.tile([128, 2], f32, tag="b2")
                nc.sync.dma_start(t_b2[:], d_b2)
                t_b3 = mcst.tile([1, C], f32, tag="b3")
                nc.sync.dma_start(t_b3[:], d_b3)
                t_sc = mcst.tile([NBOX, 1], f32, tag="score")
                nc.sync.dma_start(t_sc[:], d_sc)

                h1 = msb.tile([128, 2 * 128], f32, tag="h1")
                for cb in range(2):
                    p = mpsum.tile([128, 128], f32, tag="mp")
                    nc.tensor.matmul(p[:], t_w1b[:, cb * 128:(cb + 1) * 128],
                                     t_feat[:], start=True, stop=True)
                    nc.scalar.activation(h1[:, cb * 128:(cb + 1) * 128], p[:],
                                         mybir.ActivationFunctionType.Relu,
                                         bias=t_b1[:, cb:cb + 1], scale=1.0)
                h2 = msb.tile([128, 2 * 128], f32, tag="h2")
                for cb in range(2):
                    p = mpsum.tile([128, 128], f32, tag="mp")
                    for b in range(2):
                        nc.tensor.matmul(
                            p[:],
                            t_w2t[:, b * C + cb * 128: b * C + (cb + 1) * 128],
                            h1[:, b * 128:(b + 1) * 128],
                            start=(b == 0), stop=(b == 1))
                    nc.scalar.activation(h2[:, cb * 128:(cb + 1) * 128], p[:],
                                         mybir.ActivationFunctionType.Relu,
                                         bias=t_b2[:, cb:cb + 1], scale=1.0)
                po = mpsum.tile([128, C], f32, tag="mpo")
                for b in range(2):
                    nc.tensor.matmul(po[:], h2[:, b * 128:(b + 1) * 128],
                                     t_w3t[:, b * C:(b + 1) * C],
                                     start=(b == 0), stop=False)
                nc.tensor.matmul(po[:], t_ones1[:], t_b3[:],
                                 start=False, stop=True)
                nc.vector.tensor_scalar_mul(obj16[:], po[:], t_sc[:])

                # centered gx/gy -> edge coefficients (f32), 3-way bf16 split
                g = msb.tile([128, 8], f32, tag="gxy")
                nc.vector.tensor_scalar(
                    g[:, 0:8:2], t_pbox[:, 0:12:3], -XMIN_C, 1.0 / DX,
                    mybir.AluOpType.add, mybir.AluOpType.mult)
                nc.vector.tensor_scalar(
                    g[:, 1:8:2], t_pbox[:, 1:12:3], -YMIN_C, 1.0 / DY,
                    mybir.AluOpType.add, mybir.AluOpType.mult)
                # coef[e] = (alpha=vx, beta=-vy, gamma=vy*ax - vx*ay)
                coefF = msb.tile([128, 12], f32, tag="coefF")
                tmp = msb.tile([128, 3], f32, tag="ctmp")
                for e in range(4):
                    en = (e + 1) % 4
                    nc.vector.tensor_tensor(
                        coefF[:, 3 * e:3 * e + 1], g[:, 2 * en:2 * en + 1],
                        g[:, 2 * e:2 * e + 1], mybir.AluOpType.subtract)
                    nc.vector.tensor_tensor(
                        tmp[:, 0:1], g[:, 2 * en + 1:2 * en + 2],
                        g[:, 2 * e + 1:2 * e + 2], mybir.AluOpType.subtract)
                    nc.vector.tensor_scalar_mul(
                        coefF[:, 3 * e + 1:3 * e + 2], tmp[:, 0:1], -1.0)
                    nc.vector.tensor_tensor(
                        tmp[:, 1:2], tmp[:, 0:1], g[:, 2 * e:2 * e + 1],
                        mybir.AluOpType.mult)
                    nc.vector.tensor_tensor(
                        tmp[:, 2:3], coefF[:, 3 * e:3 * e + 1],
                        g[:, 2 * e + 1:2 * e + 2], mybir.AluOpType.mult)
                    nc.vector.tensor_tensor(
                        coefF[:, 3 * e + 2:3 * e + 3], tmp[:, 1:2],
                        tmp[:, 2:3], mybir.AluOpType.subtract)
                # 3-way bf16 split: coefB16 cols per edge:
                # [ah am al bh bm bl gh gm gl -1]
                coefB16 = msb.tile([128, 40], bf16, tag="coefB16")
                nc.vector.memset(coefB16[:, 9:40:10], -1.0)
                rem = msb.tile([128, 12], f32, tag="rem")
                rem2 = msb.tile([128, 12], f32, tag="rem2")
                hi32 = msb.tile([128, 12], f32, tag="hi32")
                # lvl 0: hi = bf16(coef); rem = coef - hi
                for cc in range(3):
                    nc.vector.tensor_copy(coefB16[:, cc:40:10],
                                          coefF[:, cc:12:3])
                    nc.vector.tensor_copy(hi32[:, cc:12:3],
                                          coefB16[:, cc:40:10])
                nc.vector.tensor_tensor(rem[:], coefF[:], hi32[:],
                                        mybir.AluOpType.subtract)
                # lvl 1: mid = bf16(rem); rem2 = rem - mid
                for cc in range(3):
                    nc.vector.tensor_copy(coefB16[:, 3 + cc:40:10],
                                          rem[:, cc:12:3])
                    nc.vector.tensor_copy(hi32[:, cc:12:3],
                                          coefB16[:, 3 + cc:40:10])
                nc.vector.tensor_tensor(rem2[:], rem[:], hi32[:],
                                        mybir.AluOpType.subtract)
                # lvl 2: lo = bf16(rem2)
                for cc in range(3):
                    nc.vector.tensor_copy(coefB16[:, 6 + cc:40:10],
                                          rem2[:, cc:12:3])
                # transpose via f32 PE path (baseline-proven), downcast after
                coefB32 = msb.tile([128, 40], f32, tag="coefB32")
                nc.vector.tensor_copy(coefB32[:], coefB16[:])
                for e in range(4):
                    pt = mpsum.tile([10, 128], f32, tag="mptr")
                    nc.tensor.transpose(pt[:], coefB32[:, 10 * e:10 * e + 10],
                                        t_eye[:])
                    ct = msb.tile([10, 128], bf16, tag="ctT")
                    nc.vector.tensor_copy(ct[:], pt[:])
                    nc.sync.dma_start(coefTall[32 * e:32 * e + 10, :], ct[:])

            # reps>1 wraps the body in a hardware loop (For_i) so test.py
            # can time many body executions in one dispatch, amortizing
            # the multi-ms axon RPC noise to nothing.  reps==1 (the
            # correctness path) emits the body directly, no loop.
            rep_cm = tc.For_i(0, reps) if reps > 1 else None
            if rep_cm is not None:
                rep_cm.__enter__()
            if True:
                # ---------------- rasterization (dense) ----------------
                with ExitStack() as rctx:
                    cr_p = rctx.enter_context(
                        tc.tile_pool(name="cross", bufs=4, space="PSUM"))
                    cnt_p = rctx.enter_context(
                        tc.tile_pool(name="cnt", bufs=1, space="PSUM"))
                    ft_p = rctx.enter_context(
                        tc.tile_pool(name="feat", bufs=2, space="PSUM"))
                    sc_p = rctx.enter_context(tc.tile_pool(name="rscr", bufs=2))
                    mk_p = rctx.enter_context(tc.tile_pool(name="mask", bufs=2))

                    for t in range(40):
                        for ri, (rc0, rw) in enumerate(plan["rspans"][t]):
                            n = 5 * rw
                            base = DOFF + (1 + 5 * t) * WL + rc0
                            goff = plan["goffs"][t][ri]
                            gt = t_gridp
                            crs = []
                            for e in range(4):
                                cr = cr_p.tile([128, n], f32, tag="cr",
                                               padded_shape=[128, 512])
                                nc.tensor.matmul(
                                    cr[:], coefTall[32 * e:32 * e + 10, :],
                                    gt[32 * e:32 * e + 10, goff:goff + n],
                                    tile_position=(32 * e, 0),
                                    start=True, stop=True)
                                crs.append(cr)
                            s = sc_p.tile([128, n], f32, tag="mins",
                                          padded_shape=[128, 512])
                            nc.scalar.copy(s[:], crs[0][:])
                            for e in range(1, 4):
                                nc.vector.tensor_tensor(
                                    s[:], s[:], crs[e][:],
                                    mybir.AluOpType.min)
                            mask = mk_p.tile([128, n], f16, tag="m",
                                             padded_shape=[128, 512])
                            nc.vector.tensor_scalar(mask[:], s[:], 0.0, None,
                                                    mybir.AluOpType.is_ge)
                            cnt = cnt_p.tile([128, n], f32, tag="c",
                                             padded_shape=[128, 512])
                            nc.tensor.matmul(cnt[:], t_ones16[:], mask[:],
                                             start=True, stop=True)
                            rin = sc_p.tile([128, n], f32, tag="rin",
                                            padded_shape=[128, 512])
                            nc.vector.tensor_scalar_max(rin[:], cnt[:], 1.0)
                            nc.vector.reciprocal_approx_fast(rin[:], rin[:])
                            msc = mk_p.tile([128, n], f16, tag="msc",
                                            padded_shape=[128, 512])
                            nc.vector.tensor_tensor(msc[:], mask[:], rin[:],
                                                    mybir.AluOpType.mult)
                            for cb in range(2):
                                ft = ft_p.tile([128, n], f32, tag="ft",
                                               padded_shape=[128, 512])
                                nc.tensor.matmul(
                                    ft[:], obj16[:, cb * 128:(cb + 1) * 128],
                                    msc[:], start=True, stop=True)
                                dst3 = bufs[0][cb][:, base:base + 5 * WL]\
                                    .rearrange("p (r c) -> p r c",
                                               r=5)[:, :, :rw]
                                ft3 = ft[:].rearrange("p (r c) -> p r c", r=5)
                                nc.scalar.copy(dst3, ft3)

                # ---------------- conv blocks (span-sparse) ----------------
                with ExitStack() as cctx:
                    w_p = cctx.enter_context(tc.tile_pool(name="cw", bufs=2))
                    cp_p = cctx.enter_context(
                        tc.tile_pool(name="cpsum", bufs=8, space="PSUM"))
                    st_p = cctx.enter_context(
                        tc.tile_pool(name="cstage", bufs=2))

                    wk_cur = w_p.tile([128, 9 * 4 * 128], f16, tag="wk",
                                      name="wk_cur")
                    nc.sync.dma_start(wk_cur[:], d_cw[0])
                    for k in range(6):
                        j = k % 2
                        wk = wk_cur
                        if k < 5:
                            # prefetch next layer's weights behind compute
                            wk_cur = w_p.tile([128, 9 * 4 * 128], f16,
                                              tag="wk", name="wk_nxt")
                            nc.sync.dma_start(wk_cur[:], d_cw[k + 1])
                        src = bufs[k % 2]
                        dst = bufs[(k + 1) % 2]
                        kspans = plan["spans"][k]
                        # background fills: cells layer k+1 reads (or k+2
                        # residual-reads) that layer k does not write
                        for cb in range(2):
                            bgb = t_bgc[:, 2 * k + cb:2 * k + cb + 1]
                            for (r0, nr, c0, w) in plan["fills"][k]:
                                base = DOFF + (1 + r0) * WL + c0
                                d3 = dst[cb][:, base:base + nr * WL]\
                                    .rearrange("p (r c) -> p r c",
                                               r=nr)[:, :, :w]
                                bb = bgb.unsqueeze(1).to_broadcast(
                                    (128, nr, w))
                                nc.vector.tensor_copy(d3, bb)
                        # rare in-body border rewrites (startup value was
                        # overwritten by a later layer's startup border)
                        for (kk, r0, nr, c0, nc_, cb, off,
                             inbody) in bord_entries:
                            if not inbody or kk != k:
                                continue
                            base = DOFF + (1 + r0) * WL + c0
                            d3 = dst[cb][:, base:base + nr * WL]\
                                .rearrange("p (r c) -> p r c",
                                           r=nr)[:, :, :nc_]
                            s3 = d_bord[:, cb * BTOT + off:
                                        cb * BTOT + off + nr * nc_]\
                                .rearrange("p (r c) -> p r c", r=nr)
                            nc.sync.dma_start(d3, s3)
                        for t in range(40):
                            for (c0, w) in kspans[t]:
                                base = DOFF + (1 + 5 * t) * WL + c0
                                for cb in range(2):
                                    ps = cp_p.tile([128, 5 * w], f32, tag="ps",
                                                   padded_shape=[128, 512])
                                    ps3 = ps[:].rearrange("p (r c) -> p r c",
                                                          r=5)
                                    idx = 0
                                    for tap in range(9):
                                        dly, dlx = tap // 3 - 1, tap % 3 - 1
                                        delta = dly * WL + dlx
                                        for ci in range(2):
                                            lh = wk[:, ((tap * 2 + ci) * 2 + cb)
                                                    * 128:
                                                    ((tap * 2 + ci) * 2 + cb
                                                     + 1) * 128]
                                            rhs = src[ci][:, base + delta:
                                                          base + delta
                                                          + 5 * WL]
                                            rhs = rhs.rearrange(
                                                "p (r c) -> p r c",
                                                r=5)[:, :, :w]
                                            nc.tensor.matmul(
                                                ps[:], lh, rhs,
                                                start=(idx == 0),
                                                stop=(idx == 17))
                                            idx += 1
                                    sc_ap = t_bns[:, 2 * k + cb:2 * k + cb + 1]
                                    bi_ap = t_bnb[:, 2 * k + cb:2 * k + cb + 1]
                                    dsl = dst[cb][:, base:base + 5 * WL]\
                                        .rearrange("p (r c) -> p r c",
                                                   r=5)[:, :, :w]
                                    if j == 0:
                                        nc.scalar.activation(
                                            dsl, ps3,
                                            mybir.ActivationFunctionType.Relu,
                                            bias=bi_ap, scale=sc_ap)
                                    else:
                                        bn = st_p.tile([128, 5 * w], f32,
                                                       tag="bn",
                                                       padded_shape=[128, 512])
                                        bn3 = bn[:].rearrange(
                                            "p (r c) -> p r c", r=5)
                                        nc.scalar.activation(
                                            bn3, ps3,
                                            mybir.ActivationFunctionType
                                            .Identity,
                                            bias=bi_ap, scale=sc_ap)
                                        nc.vector.tensor_tensor(
                                            dsl, bn3, dsl,
                                            mybir.AluOpType.add)
                                        nc.vector.tensor_scalar_max(
                                            dsl, dsl, 0.0)

                # ---------------- output DMA ----------------
                for cb in range(2):
                    for rch in range(8):
                        r0 = rch * 25
                        base = DOFF + (1 + r0) * WL + HALO
                        s3 = bufs[0][cb][:, base:base + 25 * WL].rearrange(
                            "p (r c) -> p r c", r=25)[:, :, :Wc]
                        nc.sync.dma_start(
                            d_out[cb * 128:(cb + 1) * 128, r0:r0 + 25, :],
                            s3)
            if rep_cm is not None:
                rep_cm.__exit__(None, None, None)
    nc.compile()
    return nc


# ---------------------------------------------------------------------------
# host-side input prep
# ---------------------------------------------------------------------------

def _prep_shared(pred_box, pred_score, w1, b1, w2, b2, w3, b3,
                 conv_w, bn_gamma, bn_beta, bn_mean, bn_var):
    import ml_dtypes
    f32 = np.float32
    pbox = np.ascontiguousarray(pred_box.reshape(NBOX, 24).astype(f32))
    feat = np.concatenate([pbox, pred_score.reshape(NBOX, 1).astype(f32)],
                          axis=1)
    featT26 = np.concatenate(
        [feat.T, np.ones((1, NBOX), f32)], axis=0).astype(f32)
    w1b = np.concatenate([w1.astype(f32), b1.reshape(1, C).astype(f32)],
                         axis=0)

    def two_blk(w):
        n = w.shape[1]
        o = np.empty((128, 2 * n), f32)
        o[:, :n] = w[:128]
        o[:, n:] = w[128:]
        return np.ascontiguousarray(o)

    w2t = two_blk(w2.astype(f32))
    w3t = two_blk(w3.astype(f32))
    b1s = np.ascontiguousarray(b1.astype(f32).reshape(2, 128).T)
    b2s = np.ascontiguousarray(b2.astype(f32).reshape(2, 128).T)
    b3r = b3.astype(f32).reshape(1, C)
    score = np.ascontiguousarray(pred_score.astype(f32).reshape(NBOX, 1))
    eye16 = np.eye(128, dtype=f32)

    cw = conv_w.astype(f32).reshape(6, C, C, 3, 3)
    cwt = cw.transpose(0, 3, 4, 2, 1)
    cwt = cwt.reshape(6, 9, 2, 128, 2, 128)
    cwt = cwt.transpose(0, 3, 1, 2, 4, 5)
    convw = np.ascontiguousarray(
        cwt.reshape(6, 128, 9 * 4 * 128).astype(np.float16))

    g64 = np.float64
    inv = (bn_gamma.astype(g64) / np.sqrt(bn_var.astype(g64) + BN_EPS))
    bnb = (bn_beta.astype(g64) - bn_mean.astype(g64) * inv)
    bns_ = np.empty((128, 12), f32)
    bnb_ = np.empty((128, 12), f32)
    for k in range(6):
        for cb in range(2):
            bns_[:, 2 * k + cb] = inv.reshape(6, C)[k][cb * 128:(cb + 1) * 128]
            bnb_[:, 2 * k + cb] = bnb.reshape(6, C)[k][cb * 128:(cb + 1) * 128]

    bg = _bg_consts(w1, b1, w2, b2, w3, b3, conv_w, bn_gamma, bn_beta,
                    bn_mean, bn_var)
    bgc = np.empty((128, 12), np.float16)
    for k in range(6):
        for cb in range(2):
            bgc[:, 2 * k + cb] = bg[k][cb * 128:(cb + 1) * 128]

    return dict(pbox=pbox, featT26=featT26, w1b=w1b, w2t=w2t, w3t=w3t,
                b1s=b1s, b2s=b2s, b3r=b3r, score=score, eye16=eye16,
                convw=convw, bnscale=bns_, bnbias=bnb_, bgc=bgc)


def _core_gridp(plan):
    """Packed per-rspan grid blocks, quadrant-replicated, bf16 exact."""
    import ml_dtypes
    WL = plan["WL"]
    CXC = WL // 2
    gtot = max(plan["gtot"], 4)
    g = np.zeros((128, gtot), np.float32)
    for t in range(40):
        for ri, (c0, w) in enumerate(plan["rspans"][t]):
            off = plan["goffs"][t][ri]
            n = 5 * w
            cy = (5 * t + np.arange(5) - 100).astype(np.float32)
            cx = (np.arange(c0, c0 + w) - CXC).astype(np.float32)
            blk = np.zeros((10, 5, w), np.float32)
            # rows match the coef column layout [ah bh gh am bm gm al bl gl -1]
            for lvl in range(3):
                blk[3 * lvl + 0] = cy[:, None]
                blk[3 * lvl + 1] = cx[None, :]
                blk[3 * lvl + 2] = 1.0
            fb = blk.reshape(10, n)
            for e in range(4):
                g[32 * e:32 * e + 10, off:off + n] = fb
    return np.ascontiguousarray(g.astype(ml_dtypes.bfloat16))


def _conv_np(x, w):
    f32 = np.float32
    Cc, Hh, Ww = x.shape
    xp = np.pad(x, ((0, 0), (1, 1), (1, 1)))
    cols = np.empty((9, Cc, Hh, Ww), f32)
    for t in range(9):
        dy, dx = t // 3, t % 3
        cols[t] = xp[:, dy:dy + Hh, dx:dx + Ww]
    A = cols.reshape(9 * Cc, Hh * Ww)
    Wm = w.transpose(2, 3, 1, 0).reshape(9 * Cc, w.shape[0]).astype(f32)
    return (Wm.T @ A).reshape(-1, Hh, Ww)


_FIELD_CACHE = {}


def _border_fields(conv_w, bn_gamma, bn_beta, bn_mean, bn_var):
    """Zero-input field of every layer on a 40x40 canvas.  Bit-exact for
    all border cells (edge effects reach at most 6 cells; the canvas
    interior is >= 13 from its edges).  Cached by weight bytes."""
    key = (np.asarray(conv_w, np.float32).tobytes()
           + np.asarray(bn_gamma, np.float32).tobytes()
           + np.asarray(bn_beta, np.float32).tobytes()
           + np.asarray(bn_mean, np.float32).tobytes()
           + np.asarray(bn_var, np.float32).tobytes())
    if key in _FIELD_CACHE:
        return _FIELD_CACHE[key]
    f32, g64 = np.float32, np.float64
    inv = (bn_gamma.astype(g64) / np.sqrt(bn_var.astype(g64) + BN_EPS))
    bnb = (bn_beta.astype(g64) - bn_mean.astype(g64) * inv)
    inv = inv.astype(f32)
    bnb = bnb.astype(f32)
    x = np.zeros((C, 40, 40), f32)
    fields = []
    for blk in range(3):
        res = x
        for j in range(2):
            y = _conv_np(x, conv_w[blk, j].astype(f32))
            y = y * inv[blk, j][:, None, None] + bnb[blk, j][:, None, None]
            if j == 0:
                x = np.maximum(y, 0)
            else:
                x = np.maximum(y + res, 0)
            fields.append(x.copy())
    _FIELD_CACHE[key] = fields
    return fields


def _core_bord(plan, fields):
    entries, tot = _bord_layout(plan)
    tot = max(tot, 4)
    arr = np.zeros((128, 2 * tot), np.float16)
    lo_map = plan["w0"] - HALO

    def rmap(r):
        return r if r < 20 else (39 - (H - 1 - r) if r >= H - 20 else 20)

    def cmap(c):
        return c if c < 20 else (39 - (W - 1 - c) if c >= W - 20 else 20)

    for (k, r0, nr, c0, nc_, cb, off, inbody) in entries:
        F = fields[k]
        rows = [rmap(r) for r in range(r0, r0 + nr)]
        cols = [cmap(lo_map + c) for c in range(c0, c0 + nc_)]
        vals = F[cb * 128:(cb + 1) * 128][:, rows][:, :, cols]
        arr[:, cb * tot + off: cb * tot + off + nr * nc_] = \
            vals.reshape(128, -1).astype(np.float16)
    return arr


def _prep_inputs(pred_box, pred_score, w1, b1, w2, b2, w3, b3,
                 conv_w, bn_gamma, bn_beta, bn_mean, bn_var):
    plans = _make_plans(np.asarray(pred_box))
    shared = _prep_shared(pred_box, pred_score, w1, b1, w2, b2, w3, b3,
                          conv_w, bn_gamma, bn_beta, bn_mean, bn_var)
    fields = _border_fields(np.asarray(conv_w), np.asarray(bn_gamma),
                            np.asarray(bn_beta), np.asarray(bn_mean),
                            np.asarray(bn_var))
    in_maps = [dict(shared, gridp=_core_gridp(p), bord=_core_bord(p, fields))
               for p in plans]
    return plans, in_maps


# ---------------------------------------------------------------------------
# per-core concurrent dispatch
# ---------------------------------------------------------------------------

def _make_core_fn(nc, device):
    """Compiled single-device callable for one core's program."""
    import jax
    from concourse import mybir
    from concourse.bass2jax import (_bass_exec_p, install_neuronx_cc_hook,
                                    partition_id_tensor)

    install_neuronx_cc_hook()
    partition_name = (nc.partition_id_tensor.name
                      if nc.partition_id_tensor else None)
    in_names, out_names, out_avals, zero_outs = [], [], [], []
    for alloc in nc.m.functions[0].allocations:
        if not isinstance(alloc, mybir.MemoryLocationSet):
            continue
        name = alloc.memorylocations[0].name
        if alloc.kind == "ExternalInput":
            if name != partition_name:
                in_names.append(name)
        elif alloc.kind == "ExternalOutput":
            shape = tuple(alloc.tensor_shape)
            dtype = mybir.dt.np(alloc.dtype)
            out_names.append(name)
            out_avals.append(jax.core.ShapedArray(shape, dtype))
            zero_outs.append(np.zeros(shape, dtype))
    n_params = len(in_names)
    bind_in_names = list(in_names) + list(out_names)
    if partition_name is not None:
        bind_in_names.append(partition_name)

    def _body(*args):
        operands = list(args)
        if partition_name is not None:
            operands.append(partition_id_tensor())
        outs = _bass_exec_p.bind(
            *operands,
            out_avals=tuple(out_avals),
            in_names=tuple(bind_in_names),
            out_names=tuple(out_names),
            lowering_input_output_aliases=(),
            sim_require_finite=True,
            sim_require_nnan=True,
            nc=nc,
        )
        return tuple(outs)

    fn = jax.jit(_body, keep_unused=True)
    return fn, in_names, out_names, zero_outs, device


def _dispatch_all(core_fns, in_maps):
    """Dispatch all cores async, then block; returns per-core out dicts."""
    import jax
    futs = []
    for (fn, in_names, out_names, zero_outs, device), im in zip(core_fns,
                                                                in_maps):
        args = [jax.device_put(np.asarray(im[n]), device) for n in in_names]
        args += [jax.device_put(z, device) for z in zero_outs]
        futs.append((fn(*args), out_names))
    jax.block_until_ready([f for f, _ in futs])
    return [{n: np.asarray(o) for n, o in zip(names, outs)}
            for outs, names in futs]


_CACHED = {}


def _get_programs(pred_box, reps=1):
    import jax
    key = (np.asarray(pred_box, np.float32).tobytes(), reps)
    if key not in _CACHED:
        plans = _make_plans(np.asarray(pred_box))
        devices = jax.devices()[:NCORES]
        ncs = [_build_program(p, reps=reps) for p in plans]
        core_fns = [_make_core_fn(nc, d) for nc, d in zip(ncs, devices)]
        _CACHED[key] = (plans, ncs, core_fns)
    return _CACHED[key]


def kernel(**inputs) -> np.ndarray:
    inputs = {k: np.asarray(v) for k, v in inputs.items()}
    plans, ncs, core_fns = _get_programs(inputs["pred_box"])
    _, in_maps = _prep_inputs(**inputs)
    res = _dispatch_all(core_fns, in_maps)
    out = np.empty((C, H, W), np.float32)
    for core, p in enumerate(plans):
        out[:, :, p["w0"]:p["w0"] + p["W"]] = res[core]["out"].astype(
            np.float32)
    return out


if __name__ == "__main__":
    import reference as R

    inp = {k: np.asarray(v) for k, v in R.setup_inputs().items()}
    got = kernel(**inp)
    exp = np.asarray(R.reference(**inp))
    err = np.abs(got - exp)
    rel = np.linalg.norm(got - exp) / np.linalg.norm(exp)
    print("absmax err:", err.max(), " absmax ref:", np.abs(exp).max())
    print("Relative error:", rel)

